# revision 1
# baseline (speedup 1.0000x reference)
"""Multi-head attention (B=8, N=1024, C=768, 12 heads) on 8 TRN2 NeuronCores.

Sharding: data-parallel over batch — batch element b runs on core b, weights
replicated, zero collectives.

Per-core kernel (all matmuls bf16 on the TensorEngine):
  - Host pre-transposes AND pre-converts x, W_qkv, W_proj to bf16, with
    W_qkv's q/k column blocks interleaved [q0|k0|q1|k1|...|v] so the
    first-needed weight columns are one contiguous DMA; no device casts.
  - scores are computed TRANSPOSED: S^T[k,q] with lhsT=k^T, rhs=q^T.
    Work is organized as 12 PASSES of 8 kc-steps: pass i = (head pair
    p=i//2, q-half qh=i%2). Each step's score tile [128,1024] holds
    BOTH heads' S^T chunk ([head 2p | head 2p+1] in column halves →
    separate PSUM banks): the two K=64 matmuls are issued adjacently
    and run CONCURRENTLY in PE row-groups (0,0)/(64,0), and ONE
    ScalarE exp covers the whole tile.
  - cross-pass software pipeline: pass i runs scores+exp for itself
    and P@V for pass i-1 (whose exp'd tiles are all in SBUF), so the
    ScalarE exp stream — the pacing resource — never waits for P@V or
    head boundaries.
  - softmax denominators come free: v carries a ones-column per head
    (lhsT [128,65]); row 64 of the P@V accumulator is sum_k exp(S).
  - normalization (reciprocal on DVE, broadcast on GpSimd, multiply
    reading the PSUM accumulator directly) trails each pass.
  - leftover qkv chunk emission is a queue of small matmul chains
    dispensed into the step stream as PE filler during exp waits,
    using a dedicated 2-bank PSUM pool.
  - proj: y = attn @ W_proj^T + b_proj, bias added during PSUM->SBUF
    staging; c<5 accumulation of each group runs before the c=5
    closers to absorb the last pass's normalize latency.
"""

from collections import deque
from contextlib import ExitStack

import ml_dtypes
import numpy as np

import concourse.mybir as mybir
import concourse.tile as tile
from concourse import bacc
from concourse.bass_utils import run_bass_kernel_spmd

B, N, C = 8, 1024, 768
NH, D = 12, 64
CK = C // 128  # 6 contraction chunks of 128
NQ = N // 128  # 8 position chunks of 128
SCALE = D ** -0.5
F32 = mybir.dt.float32
BF16 = mybir.dt.bfloat16
Copy = mybir.ActivationFunctionType.Copy
Exp = mybir.ActivationFunctionType.Exp
BF = ml_dtypes.bfloat16


def _emit(tc, xT, wqkvT, wprojT, bproj, out):
    nc = tc.nc
    with ExitStack() as ctx:
        sb = ctx.enter_context(tc.tile_pool(name="sb", bufs=1))
        pp = ctx.enter_context(tc.tile_pool(name="pp", bufs=11))
        small = ctx.enter_context(tc.tile_pool(name="small", bufs=2))
        stage = ctx.enter_context(tc.tile_pool(name="stage", bufs=2))
        # PSUM budget (8 banks): scores 2x[128,1024] (4 banks) + P@V
        # accumulators 3x[65,512] (3 banks, ring of 3: the draining
        # pair overlaps the active pair by one slot) + filler chain (1).
        acc = tc.alloc_tile_pool(name="acc", bufs=3, space="PSUM")
        ps = tc.alloc_tile_pool(name="ps", bufs=2, space="PSUM")
        fill = tc.alloc_tile_pool(name="fill", bufs=1, space="PSUM")

        # ---- PE warm-up ----------------------------------------------
        # Busy matmuls from ~7us until the first qk matmuls (~12.5us,
        # gated by the x + m0-weight DMAs) so HAM reaches K=8/8.
        warm_in = sb.tile([128, 512], BF16, name="warm_in", tag="warm_in")
        nc.gpsimd.memset(warm_in[:], 1.0)
        warm_ps = ps.tile([128, 512], F32, name="warm_ps", tag="s")
        # 10 matmuls cover ~6.5us..~10.8us: past the m0-weight and first
        # x-chunk DMA landings, so the first qk chains start at full
        # clock instead of hitting a HAM MID-window re-throttle.
        for i in range(10):
            nc.tensor.matmul(
                warm_ps[:],
                lhsT=warm_in[:, 0:128],
                rhs=warm_in[:],
                start=(i == 0),
                stop=(i == 9),
            )

        # ---- input DMAs (bf16, host-packed) --------------------------
        # x loads as six per-chunk tiles with plain contiguous-row
        # slices: full DMA bandwidth (the single rearranged transfer's
        # partition-major scatter ran at ~half rate), and per-chunk
        # completion lets the first qk chain's c-loop start on chunk 0.
        xT_bf = [
            sb.tile([128, N], BF16, name=f"xT_bf{c}", tag=f"xT_bf{c}")
            for c in range(CK)
        ]
        wq_bf = sb.tile([128, CK, 3 * C], BF16, name="wq_bf", tag="wq_bf")
        wp_bf = sb.tile([128, CK, C], BF16, name="wp_bf", tag="wp_bf")

        def dma_w(lo, hi):
            nc.sync.dma_start(
                out=wq_bf[:, :, lo:hi],
                in_=wqkvT[:, lo:hi].rearrange("(c p) w -> p c w", p=128),
            )

        dma_w(0, 256)  # q0|k0 -> first qk matmuls
        for c in range(CK):
            nc.sync.dma_start(
                out=xT_bf[c][:], in_=xT[c * 128:(c + 1) * 128, :]
            )
        dma_w(1536, 2304)  # v weights -> emit_v
        dma_w(256, 1536)  # remaining q/k blocks
        nc.sync.dma_start(
            out=wp_bf[:], in_=wprojT.rearrange("(c p) w -> p c w", p=128)
        )
        bp_row = sb.tile([1, C], F32, name="bp_row", tag="bp_row")
        nc.sync.dma_start(out=bp_row[:], in_=bproj[None, :])

        # ---- qkv projections (filler chains) -------------------------
        # qkT[m] m 0..5 -> q rows of heads 2m,2m+1; 6..11 -> k rows.
        # Packed weight col offset: q_m at 256m, k_m at 256m+128.
        qkT = [
            sb.tile([128, N], BF16, name=f"qkT{m}", tag=f"qkT{m}")
            for m in range(12)
        ]

        def emit_qk_half(m, qh, pool=None, ptag="f"):
            co = 256 * m if m < 6 else 256 * (m - 6) + 128
            pool = pool or fill
            qk_ps = pool.tile([128, 512], F32, name=f"qk_ps{m}_{qh}", tag=ptag)
            for c in range(CK):
                nc.tensor.matmul(
                    qk_ps[:],
                    lhsT=wq_bf[:, c, co:co + 128],
                    rhs=xT_bf[c][:, qh * 512:(qh + 1) * 512],
                    start=(c == 0),
                    stop=(c == CK - 1),
                )
            nc.vector.tensor_copy(qkT[m][:, qh * 512:(qh + 1) * 512], qk_ps[:])

        v_sb = [
            sb.tile([128, NH, D + 1], BF16, name=f"v_sb{n}", tag=f"v_sb{n}")
            for n in range(NQ)
        ]

        def emit_v_half(n, half, pool=None, ptag="f"):
            if half == 0:
                nc.gpsimd.memset(v_sb[n][:, :, D], 1.0)
            pool = pool or fill
            v_ps = pool.tile([128, 384], F32, name=f"v_ps{n}_{half}", tag=ptag)
            for c in range(CK):
                nc.tensor.matmul(
                    v_ps[:],
                    lhsT=xT_bf[c][:, n * 128:(n + 1) * 128],
                    rhs=wq_bf[:, c, 1536 + half * 384:1536 + (half + 1) * 384],
                    start=(c == 0),
                    stop=(c == CK - 1),
                )
            nc.vector.tensor_copy(
                v_sb[n][:, half * 6:(half + 1) * 6, 0:D],
                v_ps[:].rearrange("p (h d) -> p h d", d=D),
            )

        # ---- attention: 12 passes, cross-pass pipelined --------------
        attn_bf = [
            sb.tile([128, N], BF16, name=f"attn_bf{p}", tag=f"attn_bf{p}")
            for p in range(6)
        ]

        def emit_S(i, kc):
            """Score pair-tile + exp for pass i=(p,qh), chunk kc."""
            p, qh = i // 2, i % 2
            q_tile, k_tile = qkT[p], qkT[6 + p]
            qs = slice(qh * 512, (qh + 1) * 512)
            st = ps.tile([128, N], F32, name=f"st{i}_{kc}", tag="s")
            nc.tensor.matmul(
                st[:, 0:512],
                lhsT=k_tile[0:D, kc * 128:(kc + 1) * 128],
                rhs=q_tile[0:D, qs],
                start=True,
                stop=True,
            )
            nc.tensor.matmul(
                st[:, 512:1024],
                lhsT=k_tile[D:128, kc * 128:(kc + 1) * 128],
                rhs=q_tile[D:128, qs],
                start=True,
                stop=True,
            )
            pt = pp.tile([128, N], BF16, name=f"P{i}_{kc}", tag="P")
            nc.scalar.activation(pt[:], st[:], Exp, scale=SCALE)
            return pt

        def emit_pv(i, oas, kc, pt):
            """P@V chunk kc for both heads of pass i=(p,qh)."""
            p = i // 2
            for half, oa in enumerate(oas):
                nc.tensor.matmul(
                    oa[:],
                    lhsT=v_sb[kc][:, 2 * p + half, :],
                    rhs=pt[:, half * 512:(half + 1) * 512],
                    start=(kc == 0),
                    stop=(kc == NQ - 1),
                )

        def emit_norm_pre(oas):
            """Reciprocal chain for the pass's two heads (DVE/GpSimd
            only). Each denominator row (partition 64 of its
            accumulator) bounces to partition 0 because both
            reciprocal_approx_fast and partition_broadcast need
            partition-0 sources."""
            bc = []
            for half in range(2):
                dn = small.tile([1, 512], F32, name=f"dn{half}", tag=f"dn{half}")
                nc.vector.tensor_copy(dn[:], oas[half][D:D + 1, :])
                rc = small.tile([1, 512], F32, name=f"rc{half}", tag=f"rc{half}")
                nc.vector.reciprocal_approx_fast(rc[:], dn[:])
                rcb = small.tile([1, 512], BF16, name=f"rcb{half}", tag=f"rcb{half}")
                nc.vector.tensor_copy(rcb[:], rc[:])
                b = small.tile([64, 512], BF16, name=f"bc{half}", tag=f"bc{half}")
                nc.gpsimd.partition_broadcast(b[:], rcb[:])
                bc.append(b)
            return bc

        def emit_norm_post(i, oas, bc):
            p, qh = i // 2, i % 2
            qs = slice(qh * 512, (qh + 1) * 512)
            for half in range(2):
                ro = half * 64
                nc.vector.tensor_mul(
                    attn_bf[p][ro:ro + 64, qs], oas[half][0:D, :], bc[half][:]
                )

        # Pre-attention: minimum qkv to start pass 0 (+ qk(1), needed
        # by pass 2 alongside qk(7) which the fill queue delivers).
        # These chains rotate through the TWO-deep ps ring (idle until
        # the first score tile) so consecutive chains pipeline instead
        # of serializing on the one-buffer fill pool's staging copy.
        for m in (0, 6, 1):
            emit_qk_half(m, 0, pool=ps, ptag="s")
            emit_qk_half(m, 1, pool=ps, ptag="s")
        emit_v_half(0, 0, pool=ps, ptag="s")
        emit_v_half(0, 1, pool=ps, ptag="s")

        # Fill queue, deadline-ordered: v(kc) is consumed by P@V of
        # pass 0 (running during pass 1) at step 8+kc; qk(7) by pass 2
        # (step 16); qk(m),qk(6+m) by pass 2m (step 16m). Passes 0-1
        # dispense one chain per step (16 slots: 14 v + qk(7)), later
        # passes one per 3 steps.
        fills = deque()
        for n in range(1, NQ):
            fills.append(lambda n=n: emit_v_half(n, 0))
            fills.append(lambda n=n: emit_v_half(n, 1))
        for m in (7, 2, 8, 3, 9, 4, 10, 5, 11):
            fills.append(lambda m=m: emit_qk_half(m, 0))
            fills.append(lambda m=m: emit_qk_half(m, 1))

        NPASS = 12
        prev = None  # (i, pts) of previous pass (P@V pending)
        pend_post = None  # (i, oas, bc) awaiting norm_post
        for i in range(NPASS):
            if prev is not None:
                pi, ppts = prev
                # accumulators allocated lazily, at P@V time: the ring
                # slot being reused was freed by the norm_post emitted
                # at the top of this pass's first step.
                poas = (
                    acc.tile([D + 1, 512], F32, name=f"oaA{pi}", tag="acc"),
                    acc.tile([D + 1, 512], F32, name=f"oaB{pi}", tag="acc"),
                )
            pts = {}
            for kc in range(NQ):
                if kc == 0 and pend_post is not None:
                    emit_norm_post(*pend_post)
                    pend_post = None
                # dependency-free filler first, then the score pair
                # (whose exp enables the NEXT step — it must never sit
                # behind a potentially-stalled P@V), then P@V of the
                # previous pass, lagged one step: the ring slot it
                # reuses was freed by the norm_post at this pass's
                # step 0, which gets a full step to drain.
                if fills and (i < 2 or kc % 3 == 1):
                    fills.popleft()()
                pts[kc] = emit_S(i, kc)
                if prev is not None and kc >= 1:
                    emit_pv(pi, poas, kc - 1, ppts.pop(kc - 1))
            if prev is not None:
                # trailing chunk lands at the boundary, where the PE
                # would otherwise idle waiting for this pass's last exp
                emit_pv(pi, poas, NQ - 1, ppts.pop(NQ - 1))
                pend_post = (pi, poas, emit_norm_pre(poas))
            prev = (i, pts)

        # epilogue: P@V + normalize of the last pass, interleaved with
        # the projection's c<5 accumulation (pairs 0-4 only, all
        # normalized long ago). ps+fill release 5 banks for yps.
        li, lpts = prev
        loas = (
            acc.tile([D + 1, 512], F32, name=f"oaA{li}", tag="acc"),
            acc.tile([D + 1, 512], F32, name=f"oaB{li}", tag="acc"),
        )
        while fills:
            fills.popleft()()
        fill.release()
        ps.release()
        yps = tc.alloc_tile_pool(name="yps", bufs=2, space="PSUM")

        bias_bc = sb.tile([128, C], F32, name="bias_bc", tag="bias_bc")
        nc.gpsimd.partition_broadcast(bias_bc[:], bp_row[:])

        def proj_open(n):
            y_ps = yps.tile([128, C], F32, name=f"y_ps{n}", tag="y_ps")
            for lo, hi in ((0, 512), (512, 768)):
                for c in range(CK - 1):
                    nc.tensor.matmul(
                        y_ps[:, lo:hi],
                        lhsT=attn_bf[c][:, n * 128:(n + 1) * 128],
                        rhs=wp_bf[:, c, lo:hi],
                        start=(c == 0),
                        stop=False,
                    )
            return y_ps

        def proj_close(n, y_ps):
            for lo, hi in ((0, 512), (512, 768)):
                nc.tensor.matmul(
                    y_ps[:, lo:hi],
                    lhsT=attn_bf[CK - 1][:, n * 128:(n + 1) * 128],
                    rhs=wp_bf[:, CK - 1, lo:hi],
                    start=False,
                    stop=True,
                )
            y_sb = stage.tile([128, C], F32, name=f"y_sb{n}", tag="y")
            nc.vector.tensor_add(y_sb[:], y_ps[:], bias_bc[:])
            nc.sync.dma_start(out=out[n * 128:(n + 1) * 128, :], in_=y_sb[:])

        for kc in range(NQ):
            if kc == 0 and pend_post is not None:
                emit_norm_post(*pend_post)
                pend_post = None
            emit_pv(li, loas, kc, lpts.pop(kc))
        y_prev = proj_open(0)
        y_cur = proj_open(1)
        emit_norm_post(li, loas, emit_norm_pre(loas))
        for n in range(NQ):
            proj_close(n, y_prev)
            y_prev = y_cur
            if n + 2 < NQ:
                y_cur = proj_open(n + 2)
        yps.release()
        acc.release()


def build_graph():
    nc = bacc.Bacc("TRN2", target_bir_lowering=False, debug=False)
    xT = nc.declare_dram_parameter("xT", [C, N], BF16, isOutput=False)
    wqkvT = nc.declare_dram_parameter("wqkvT", [C, 3 * C], BF16, isOutput=False)
    wprojT = nc.declare_dram_parameter("wprojT", [C, C], BF16, isOutput=False)
    bproj = nc.declare_dram_parameter("bproj", [C], F32, isOutput=False)
    out = nc.declare_dram_parameter("out", [N, C], F32, isOutput=True)
    with tile.TileContext(nc) as tc:
        _emit(tc, xT.ap(), wqkvT.ap(), wprojT.ap(), bproj.ap(), out.ap())
    nc.compile()
    return nc


_GRAPH = None


def _get_graph():
    global _GRAPH
    if _GRAPH is None:
        _GRAPH = build_graph()
    return _GRAPH


def make_in_maps(x, W_qkv, W_proj, b_proj):
    x = np.asarray(x, dtype=np.float32)
    wq = np.asarray(W_qkv, dtype=np.float32).T  # [C, 3C]; cols q|k|v
    # pack q/k column blocks interleaved: [q0|k0|q1|k1|...|q5|k5|v]
    packed = np.empty((C, 3 * C), dtype=BF)
    for m in range(6):
        packed[:, 256 * m:256 * m + 128] = wq[:, 128 * m:128 * (m + 1)]
        packed[:, 256 * m + 128:256 * (m + 1)] = wq[:, C + 128 * m:C + 128 * (m + 1)]
    packed[:, 1536:] = wq[:, 1536:]
    wprojT = np.ascontiguousarray(np.asarray(W_proj, dtype=np.float32).T.astype(BF))
    bp = np.ascontiguousarray(np.asarray(b_proj, dtype=np.float32))
    xT_all = np.ascontiguousarray(x.transpose(0, 2, 1).astype(BF))
    return [
        {"xT": xT_all[i], "wqkvT": packed, "wprojT": wprojT, "bproj": bp}
        for i in range(B)
    ]


def run(x, W_qkv, W_proj, b_proj, trace=False):
    nc = _get_graph()
    in_maps = make_in_maps(x, W_qkv, W_proj, b_proj)
    res = run_bass_kernel_spmd(nc, in_maps, core_ids=list(range(B)), trace=trace)
    out = np.stack([res.results[i]["out"] for i in range(B)], axis=0)
    return out.astype(np.float32, copy=False), res


def kernel(x, W_qkv, W_proj, b_proj, H=None, W=None):
    out, _ = run(x, W_qkv, W_proj, b_proj)
    return out



# revision 6
# speedup vs baseline: 1.1889x; 1.1889x over previous
"""Multi-head attention (B=8, N=1024, C=768, 12 heads) on 8 TRN2 NeuronCores.

Sharding: data-parallel over batch — batch element b runs on core b, weights
replicated, zero collectives.

Per-core kernel (all matmuls bf16 on the TensorEngine). Structure relative
to the measured HW model:
  - per-MM floor ~256ns (N=512 stream 213ns + ~43ns issue/sem overhead);
  - a 64-row-tiled K=64 score pair runs CONCURRENTLY in ~295ns (measured),
    but every (64,128)<->(128,128) tiling-mode change costs a ~113ns drain;
  - ScalarE exp is (N+352)/1.2 ns -> ~1300ns per [128,1024] step, 125us
    total for 96 steps — slightly below the PE's total work, so the loop
    is PE-paced and every PE cycle saved is wall time.

Design:
  - scores are computed TRANSPOSED, S^T[k,q], lhsT=k^T, rhs=q^T, as 12
    passes (head pair p=i//2, q-half qh=i%2) of 4 BLOCKS of 2 kc-steps.
    Each block issues its 2 steps' score pairs back-to-back in 64-row
    tiled mode (4 MMs, 2 mode switches per block instead of 4), then an
    untiled segment: P@V of the PREVIOUS pass (2 steps' worth, evenly
    spread), plus one or two filler chains.
  - exp (ScalarE) is issued right after each score pair; softmax
    denominators come free via a ones-column in v (row 64 of the P@V
    accumulator).
  - qkv projection runs as filler chains dispensed into the untiled
    segments, deadline-ordered; only q/k of head pair 0 is computed in
    the prologue (interleaved with the x-chunk DMAs, chunk by chunk).
    DMA order: Wqkv[q0k0] -> x -> Wqkv[v] -> Wqkv[q1k1] -> rest -> Wproj,
    so no chain ever waits on a late weight block.
  - projection y = attn @ W_proj^T + b_proj is split per 128-row chunk
    into a lo [*,512] and hi [*,256] region, each accumulated in TWO
    PSUM sessions: A = c0..2 (dispensed into passes 7-9), B = c3..5
    (pass 11 / epilogue); bias is added by DVE during A's PSUM->SBUF
    staging and the final y = A_sb + B_ps add lands in the output stage
    tile. No tile-pool release barriers anywhere.
"""

from collections import deque
from contextlib import ExitStack

import ml_dtypes
import numpy as np

import concourse.mybir as mybir
import concourse.tile as tile
from concourse import bacc
from concourse.bass_utils import run_bass_kernel_spmd

B, N, C = 8, 1024, 768
NH, D = 12, 64
CK = C // 128  # 6 contraction chunks of 128
NQ = N // 128  # 8 position chunks of 128
SCALE = D ** -0.5
F32 = mybir.dt.float32
BF16 = mybir.dt.bfloat16
Copy = mybir.ActivationFunctionType.Copy
Exp = mybir.ActivationFunctionType.Exp
BF = ml_dtypes.bfloat16


def _emit(tc, xT, wqkvT, wprojT, bproj, out):
    nc = tc.nc
    with ExitStack() as ctx:
        sb = ctx.enter_context(tc.tile_pool(name="sb", bufs=1))
        pp = ctx.enter_context(tc.tile_pool(name="pp", bufs=11))
        small = ctx.enter_context(tc.tile_pool(name="small", bufs=2))
        stage = ctx.enter_context(tc.tile_pool(name="stage", bufs=2))
        # PSUM budget (8 banks): score ring 2x[128,1024] (4 banks) + P@V
        # accumulator ring 3x[65,512] (3 banks) + filler/proj chain (1).
        acc = tc.alloc_tile_pool(name="acc", bufs=3, space="PSUM")
        ps = tc.alloc_tile_pool(name="ps", bufs=2, space="PSUM")
        fill = tc.alloc_tile_pool(name="fill", bufs=1, space="PSUM")

        # ---- PE warm-up ----------------------------------------------
        # ~3us of matmuls so the HAM clock-gate opens (K=8/8) before the
        # first qk chains; the chains themselves keep it open.
        warm_in = sb.tile([128, 512], BF16, name="warm_in", tag="warm_in")
        nc.gpsimd.memset(warm_in[:], 1.0)
        warm_ps = ps.tile([128, 512], F32, name="warm_ps", tag="s")
        for i in range(12):
            nc.tensor.matmul(
                warm_ps[:],
                lhsT=warm_in[:, 0:128],
                rhs=warm_in[:],
                start=(i == 0),
                stop=(i == 11),
            )

        # ---- input DMAs (bf16, host-packed) --------------------------
        xT_bf = [
            sb.tile([128, N], BF16, name=f"xT_bf{c}", tag=f"xT_bf{c}")
            for c in range(CK)
        ]
        wq_bf = sb.tile([128, CK, 3 * C], BF16, name="wq_bf", tag="wq_bf")
        wp_bf = sb.tile([128, CK, C], BF16, name="wp_bf", tag="wp_bf")

        def dma_w(lo, hi):
            nc.sync.dma_start(
                out=wq_bf[:, :, lo:hi],
                in_=wqkvT[:, lo:hi].rearrange("(c p) w -> p c w", p=128),
            )

        dma_w(0, 256)  # q0|k0 -> prologue chains
        for c in range(CK):
            nc.sync.dma_start(
                out=xT_bf[c][:], in_=xT[c * 128:(c + 1) * 128, :]
            )
        dma_w(1536, 2304)  # v weights -> v filler chains (pass 0)
        dma_w(256, 512)  # q1|k1 -> qk(1)/qk(7) fillers (passes 0-1)
        dma_w(512, 1536)  # remaining q/k blocks
        nc.sync.dma_start(
            out=wp_bf[:], in_=wprojT.rearrange("(c p) w -> p c w", p=128)
        )
        bp_row = sb.tile([1, C], F32, name="bp_row", tag="bp_row")
        nc.sync.dma_start(out=bp_row[:], in_=bproj[None, :])

        # ---- qkv projections (filler chains) -------------------------
        # qkT[m] m 0..5 -> q rows of heads 2m,2m+1; 6..11 -> k rows.
        # Packed weight col offset: q_m at 256m, k_m at 256m+128.
        qkT = [
            sb.tile([128, N], BF16, name=f"qkT{m}", tag=f"qkT{m}")
            for m in range(12)
        ]

        def emit_qk_half(m, qh, pool=None, ptag="f"):
            co = 256 * m if m < 6 else 256 * (m - 6) + 128
            pool = pool or fill
            qk_ps = pool.tile([128, 512], F32, name=f"qk_ps{m}_{qh}", tag=ptag)
            for c in range(CK):
                nc.tensor.matmul(
                    qk_ps[:],
                    lhsT=wq_bf[:, c, co:co + 128],
                    rhs=xT_bf[c][:, qh * 512:(qh + 1) * 512],
                    start=(c == 0),
                    stop=(c == CK - 1),
                )
            nc.vector.tensor_copy(qkT[m][:, qh * 512:(qh + 1) * 512], qk_ps[:])

        v_sb = [
            sb.tile([128, NH, D + 1], BF16, name=f"v_sb{n}", tag=f"v_sb{n}")
            for n in range(NQ)
        ]

        def emit_v_half(n, half, pool=None, ptag="f"):
            if half == 0:
                nc.gpsimd.memset(v_sb[n][:, :, D], 1.0)
            pool = pool or fill
            v_ps = pool.tile([128, 384], F32, name=f"v_ps{n}_{half}", tag=ptag)
            for c in range(CK):
                nc.tensor.matmul(
                    v_ps[:],
                    lhsT=xT_bf[c][:, n * 128:(n + 1) * 128],
                    rhs=wq_bf[:, c, 1536 + half * 384:1536 + (half + 1) * 384],
                    start=(c == 0),
                    stop=(c == CK - 1),
                )
            nc.vector.tensor_copy(
                v_sb[n][:, half * 6:(half + 1) * 6, 0:D],
                v_ps[:].rearrange("p (h d) -> p h d", d=D),
            )

        # ---- prologue: q/k of head pair 0, DMA-pipelined -------------
        # (0,0) gives q positions 0:512 (pass 0 is qh=0); (6,0)/(6,1)
        # give k positions 0:512 / 512:1024. Interleave each pair's
        # 6-MM chains by c-chunk so MMs start as x chunks land.
        def emit_qk_pair(specA, specB):
            (mA, qA), (mB, qB) = specA, specB
            coA = 256 * mA if mA < 6 else 256 * (mA - 6) + 128
            coB = 256 * mB if mB < 6 else 256 * (mB - 6) + 128
            pA = ps.tile([128, 512], F32, name=f"pre{mA}_{qA}", tag="s")
            pB = ps.tile([128, 512], F32, name=f"pre{mB}_{qB}", tag="s")
            for c in range(CK):
                nc.tensor.matmul(
                    pA[:], lhsT=wq_bf[:, c, coA:coA + 128],
                    rhs=xT_bf[c][:, qA * 512:(qA + 1) * 512],
                    start=(c == 0), stop=(c == CK - 1),
                )
                nc.tensor.matmul(
                    pB[:], lhsT=wq_bf[:, c, coB:coB + 128],
                    rhs=xT_bf[c][:, qB * 512:(qB + 1) * 512],
                    start=(c == 0), stop=(c == CK - 1),
                )
            nc.vector.tensor_copy(qkT[mA][:, qA * 512:(qA + 1) * 512], pA[:])
            nc.vector.tensor_copy(qkT[mB][:, qB * 512:(qB + 1) * 512], pB[:])

        emit_qk_pair((0, 0), (6, 0))
        emit_qk_pair((6, 1), (0, 1))

        # ---- attention: 12 passes x 4 blocks of 2 steps --------------
        attn_bf = [
            sb.tile([128, N], BF16, name=f"attn_bf{p}", tag=f"attn_bf{p}")
            for p in range(6)
        ]

        def emit_S(i, kc):
            """Row-tiled score pair + exp for pass i=(p,qh), chunk kc."""
            p, qh = i // 2, i % 2
            q_tile, k_tile = qkT[p], qkT[6 + p]
            qs = slice(qh * 512, (qh + 1) * 512)
            st = ps.tile([128, N], F32, name=f"st{i}_{kc}", tag="s")
            nc.tensor.matmul(
                st[:, 0:512],
                lhsT=k_tile[0:D, kc * 128:(kc + 1) * 128],
                rhs=q_tile[0:D, qs],
                start=True,
                stop=True,
            )
            nc.tensor.matmul(
                st[:, 512:1024],
                lhsT=k_tile[D:128, kc * 128:(kc + 1) * 128],
                rhs=q_tile[D:128, qs],
                start=True,
                stop=True,
            )
            pt = pp.tile([128, N], BF16, name=f"P{i}_{kc}", tag="P")
            nc.scalar.activation(pt[:], st[:], Exp, scale=SCALE)
            return pt

        def emit_pv(i, oas, kc, pt):
            """P@V chunk kc for both heads of pass i=(p,qh)."""
            p = i // 2
            for half, oa in enumerate(oas):
                nc.tensor.matmul(
                    oa[:],
                    lhsT=v_sb[kc][:, 2 * p + half, :],
                    rhs=pt[:, half * 512:(half + 1) * 512],
                    start=(kc == 0),
                    stop=(kc == NQ - 1),
                )

        def emit_norm_pre(oas):
            """Reciprocal chain for the pass's two heads (DVE/GpSimd)."""
            bc = []
            for half in range(2):
                dn = small.tile([1, 512], F32, name=f"dn{half}", tag=f"dn{half}")
                nc.vector.tensor_copy(dn[:], oas[half][D:D + 1, :])
                rc = small.tile([1, 512], F32, name=f"rc{half}", tag=f"rc{half}")
                nc.vector.reciprocal_approx_fast(rc[:], dn[:])
                rcb = small.tile([1, 512], BF16, name=f"rcb{half}", tag=f"rcb{half}")
                nc.vector.tensor_copy(rcb[:], rc[:])
                b = small.tile([64, 512], BF16, name=f"bc{half}", tag=f"bc{half}")
                nc.gpsimd.partition_broadcast(b[:], rcb[:])
                bc.append(b)
            return bc

        def emit_norm_post(i, oas, bc):
            p, qh = i // 2, i % 2
            qs = slice(qh * 512, (qh + 1) * 512)
            for half in range(2):
                ro = half * 64
                nc.vector.tensor_mul(
                    attn_bf[p][ro:ro + 64, qs], oas[half][0:D, :], bc[half][:]
                )

        # ---- projection chains (sessions A: c0-2+bias, B: c3-5) ------
        bias_bc = sb.tile([128, C], F32, name="bias_bc", tag="bias_bc")
        nc.gpsimd.partition_broadcast(bias_bc[:], bp_row[:])
        regions = [(0, 512), (512, 768)]
        projA_sb = [
            [
                sb.tile([128, hi - lo], F32, name=f"pA{n}_{r}", tag=f"pA{n}_{r}")
                for r, (lo, hi) in enumerate(regions)
            ]
            for n in range(NQ)
        ]
        y_stage = [None] * NQ

        def emit_projA(n, r, pool=None, ptag="f"):
            lo, hi = regions[r]
            pool = pool or fill
            a_ps = pool.tile([128, hi - lo], F32, name=f"aps{n}_{r}", tag=ptag)
            for c in range(3):
                nc.tensor.matmul(
                    a_ps[:],
                    lhsT=attn_bf[c][:, n * 128:(n + 1) * 128],
                    rhs=wp_bf[:, c, lo:hi],
                    start=(c == 0),
                    stop=(c == 2),
                )
            nc.vector.tensor_add(projA_sb[n][r][:], a_ps[:], bias_bc[:, lo:hi])

        def emit_projB(n, r, pool=None, ptag="f"):
            lo, hi = regions[r]
            pool = pool or fill
            b_ps = pool.tile([128, hi - lo], F32, name=f"bps{n}_{r}", tag=ptag)
            for c in range(3, CK):
                nc.tensor.matmul(
                    b_ps[:],
                    lhsT=attn_bf[c][:, n * 128:(n + 1) * 128],
                    rhs=wp_bf[:, c, lo:hi],
                    start=(c == 3),
                    stop=(c == CK - 1),
                )
            if y_stage[n] is None:
                y_stage[n] = stage.tile([128, C], F32, name=f"y{n}", tag="y")
            nc.vector.tensor_add(y_stage[n][:, lo:hi], b_ps[:], projA_sb[n][r][:])
            if r == 1:
                nc.sync.dma_start(
                    out=out[n * 128:(n + 1) * 128, :], in_=y_stage[n][:]
                )

        # ---- fill queues ---------------------------------------------
        # fills: deadline-ordered qkv chains. v(n,0) consumed by P@V of
        # pass 0 (during pass 1, block n//2); qk(m)/qk(6+m) by pass 2m;
        # v(n,1) first consumed by P@V of pass 6 (during pass 7).
        fills = deque()
        for n in range(NQ):
            fills.append(lambda n=n: emit_v_half(n, 0))
        for m in (7, 1):
            fills.append(lambda m=m: emit_qk_half(m, 0))
        for m in (7, 1):
            fills.append(lambda m=m: emit_qk_half(m, 1))
        for m in (8, 2):
            fills.append(lambda m=m: emit_qk_half(m, 0))
        for m in (8, 2):
            fills.append(lambda m=m: emit_qk_half(m, 1))
        for m in (9, 3):
            fills.append(lambda m=m: emit_qk_half(m, 0))
        for m in (9, 3):
            fills.append(lambda m=m: emit_qk_half(m, 1))
        for n in range(NQ):
            fills.append(lambda n=n: emit_v_half(n, 1))
        for m in (10, 4):
            fills.append(lambda m=m: emit_qk_half(m, 0))
        for m in (10, 4):
            fills.append(lambda m=m: emit_qk_half(m, 1))
        for m in (11, 5):
            fills.append(lambda m=m: emit_qk_half(m, 0))
        for m in (11, 5):
            fills.append(lambda m=m: emit_qk_half(m, 1))

        # projection session-A chains: need pairs 0-2 normalized (end of
        # pass 6); dispensed during passes 7-9. Session-B n<4 chains
        # need pair5 qh0 normalized (pass 11 block 1); dispensed late in
        # pass 11. Session-B n>=4 runs in the epilogue.
        projA_q = deque(
            (n, r) for n in range(NQ) for r in range(2)
        )
        projB_q = deque((n, r) for n in range(4) for r in range(2))

        NPASS = 12
        prev = None  # (i, pts) of previous pass (P@V pending this pass)
        pend_post = None  # (i, oas, bc) awaiting norm_post
        for i in range(NPASS):
            if prev is not None:
                pi, ppts = prev
                poas = (
                    acc.tile([D + 1, 512], F32, name=f"oaA{pi}", tag="acc"),
                    acc.tile([D + 1, 512], F32, name=f"oaB{pi}", tag="acc"),
                )
            pts = {}
            for b in range(4):
                # tiled segment: this block's two score pairs + exps
                pts[2 * b] = emit_S(i, 2 * b)
                pts[2 * b + 1] = emit_S(i, 2 * b + 1)
                # untiled segment
                if b == 0 and pend_post is not None:
                    emit_norm_post(*pend_post)
                    pend_post = None
                if prev is not None:
                    emit_pv(pi, poas, 2 * b, ppts.pop(2 * b))
                    emit_pv(pi, poas, 2 * b + 1, ppts.pop(2 * b + 1))
                # filler dispensing (deadline-ordered)
                nfill = 2 if i < 2 else 1
                for _ in range(nfill):
                    if fills:
                        fills.popleft()()
                if not fills and i >= 7 and projA_q:
                    emit_projA(*projA_q.popleft())
                    if projA_q and b % 2 == 1:
                        emit_projA(*projA_q.popleft())
            if prev is not None:
                pend_post = (pi, poas, emit_norm_pre(poas))
            prev = (i, pts)

        # ---- epilogue ------------------------------------------------
        # P@V + normalize of pass 11, remaining session-A (shouldn't
        # happen) and session-B chains, final adds + output DMA.
        li, lpts = prev
        loas = (
            acc.tile([D + 1, 512], F32, name=f"oaA{li}", tag="acc"),
            acc.tile([D + 1, 512], F32, name=f"oaB{li}", tag="acc"),
        )
        while projA_q:
            emit_projA(*projA_q.popleft())
        for kc in range(NQ):
            if kc == 2 and pend_post is not None:
                # norm_post(pass 10) = pair 5 qh0; also frees an acc slot
                emit_norm_post(*pend_post)
                pend_post = None
            emit_pv(li, loas, kc, lpts.pop(kc))
            # session-B for n<4 reads attn_bf[5] qh0 columns, written by
            # norm_post(10) above at kc==2 — only dispense after that.
            if kc >= 3 and projB_q:
                emit_projB(*projB_q.popleft())
        while projB_q:
            emit_projB(*projB_q.popleft())
        emit_norm_post(li, loas, emit_norm_pre(loas))
        # n>=4 session-B: pair5 qh1 now normalized; ping-pong across the
        # acc ring (3 banks) + fill bank so chains overlap their adds.
        for j, (n, r) in enumerate([(n, r) for n in range(4, NQ) for r in range(2)]):
            if j % 4 == 3:
                emit_projB(n, r, pool=fill, ptag="f")
            else:
                emit_projB(n, r, pool=acc, ptag="acc")
        fill.release()
        ps.release()
        acc.release()


def build_graph():
    nc = bacc.Bacc("TRN2", target_bir_lowering=False, debug=False)
    xT = nc.declare_dram_parameter("xT", [C, N], BF16, isOutput=False)
    wqkvT = nc.declare_dram_parameter("wqkvT", [C, 3 * C], BF16, isOutput=False)
    wprojT = nc.declare_dram_parameter("wprojT", [C, C], BF16, isOutput=False)
    bproj = nc.declare_dram_parameter("bproj", [C], F32, isOutput=False)
    out = nc.declare_dram_parameter("out", [N, C], F32, isOutput=True)
    with tile.TileContext(nc) as tc:
        _emit(tc, xT.ap(), wqkvT.ap(), wprojT.ap(), bproj.ap(), out.ap())
    nc.compile()
    return nc


_GRAPH = None


def _get_graph():
    global _GRAPH
    if _GRAPH is None:
        _GRAPH = build_graph()
    return _GRAPH


def make_in_maps(x, W_qkv, W_proj, b_proj):
    x = np.asarray(x, dtype=np.float32)
    wq = np.asarray(W_qkv, dtype=np.float32).T  # [C, 3C]; cols q|k|v
    # pack q/k column blocks interleaved: [q0|k0|q1|k1|...|q5|k5|v]
    packed = np.empty((C, 3 * C), dtype=BF)
    for m in range(6):
        packed[:, 256 * m:256 * m + 128] = wq[:, 128 * m:128 * (m + 1)]
        packed[:, 256 * m + 128:256 * (m + 1)] = wq[:, C + 128 * m:C + 128 * (m + 1)]
    packed[:, 1536:] = wq[:, 1536:]
    wprojT = np.ascontiguousarray(np.asarray(W_proj, dtype=np.float32).T.astype(BF))
    bp = np.ascontiguousarray(np.asarray(b_proj, dtype=np.float32))
    xT_all = np.ascontiguousarray(x.transpose(0, 2, 1).astype(BF))
    return [
        {"xT": xT_all[i], "wqkvT": packed, "wprojT": wprojT, "bproj": bp}
        for i in range(B)
    ]


def run(x, W_qkv, W_proj, b_proj, trace=False):
    nc = _get_graph()
    in_maps = make_in_maps(x, W_qkv, W_proj, b_proj)
    res = run_bass_kernel_spmd(nc, in_maps, core_ids=list(range(B)), trace=trace)
    out = np.stack([res.results[i]["out"] for i in range(B)], axis=0)
    return out.astype(np.float32, copy=False), res


def kernel(x, W_qkv, W_proj, b_proj, H=None, W=None):
    out, _ = run(x, W_qkv, W_proj, b_proj)
    return out
